# revision 1
# baseline (speedup 1.0000x reference)
"""Fused tensor-parallel transformer layer for Trainium2 (8 NeuronCores).

Sharding: Megatron-style tensor parallel. Each core owns 4 heads of the
attention block (q/k/v projection rows, o_proj columns) and 1/8 of the FFN
hidden dim (w1 rows, w2 columns). LayerNorms are computed replicated on
every core. One on-device AllReduce joins the attention block to the FFN
block; the final residual sum is assembled on the host from per-core
partial outputs (each core adds x2/8 so the partials sum to the answer).

All activations live transposed ([hid, seq]) so every matmul contracts
over the partition dim with zero on-device transposes. Matmuls run in
float32r (fp32 operands truncated to ~fp22 inside the PE) which is
full-rate on TRN2 for moving dims >= 256.
"""

import math
import ml_dtypes
import numpy as np

import concourse.bass as bass
import concourse.mybir as mybir
import concourse.tile as tile
from concourse import bacc
from concourse.bass_utils import run_bass_kernel_spmd
from concourse.masks import make_identity

FP = mybir.dt.float32
BF = mybir.dt.bfloat16
P = 128
EPS = 1e-6
AF = mybir.ActivationFunctionType
ALU = mybir.AluOpType


def fr(ap):
    return ap.bitcast(mybir.dt.float32r)


CFG_FULL = dict(
    seq=2048, hid=4096, ffn=16384, n_cores=8, n_heads=32,
    d_nope=128, d_rope=64, d_v=128, sb=512, fb=1024,
)


def build_layer_kernel(cfg, mask_mode, ln1_affine, ln2_affine):
    """mask_mode: 'causal' (skip tiles above diag, add mask on diag tiles),
    'zero' (no mask at all), 'full' (add mask everywhere)."""
    seq, hid, ffn = cfg["seq"], cfg["hid"], cfg["ffn"]
    n_cores, n_heads = cfg["n_cores"], cfg["n_heads"]
    d_nope, d_rope, d_v = cfg["d_nope"], cfg["d_rope"], cfg["d_v"]
    SB, FB = cfg["sb"], cfg["fb"]
    half = d_rope // 2
    hpc = n_heads // n_cores              # heads per core
    nkt = hid // P                        # hid k-tiles
    nsb = seq // SB                       # attention s-blocks
    sbt = SB // P                         # sk tiles per s-block
    nskt = seq // P                       # total sk tiles
    n_rope_ot = hpc * d_rope // P         # rope o-tiles (2 heads each)
    qo = hpc * d_nope // P + n_rope_ot    # q/k o-tiles per core
    dvc = hpc * d_v                       # v cols per core
    ndvt = dvc // P                       # o_proj contraction tiles
    fpc = ffn // n_cores                  # ffn rows per core
    nft = fpc // P                        # f tiles per core
    nfb = seq // FB                       # ffn s-blocks
    nsub = FB // SB                       # 512-wide sub blocks per ffn block
    assert hpc % 2 == 0 and half == 32 and d_nope == P and d_v == P

    nc = bacc.Bacc(None, target_bir_lowering=False)

    xt_d = nc.dram_tensor("xt", [hid, seq], FP, kind="ExternalInput")
    wq_d = nc.dram_tensor("wq_t", [nkt, qo, P, P], FP, kind="ExternalInput")
    wk_d = nc.dram_tensor("wk_t", [nkt, qo, P, P], FP, kind="ExternalInput")
    wv_d = nc.dram_tensor("wv_t", [nkt, P, dvc], FP, kind="ExternalInput")
    wo_d = nc.dram_tensor("wo_t", [ndvt, nkt, P, P], FP, kind="ExternalInput")
    w1_d = nc.dram_tensor("w1_t", [nkt, nft, P, P], FP, kind="ExternalInput")
    w2_d = nc.dram_tensor("w2_t", [nft, nkt, P, P], BF, kind="ExternalInput")
    cos_d = nc.dram_tensor("cos_t", [P, seq], FP, kind="ExternalInput")
    sin_d = nc.dram_tensor("sin_t", [P, seq], FP, kind="ExternalInput")
    rp_d = nc.dram_tensor("rperm", [P, P], FP, kind="ExternalInput")
    ones_d = nc.dram_tensor("ones_d", [P, P], FP, kind="ExternalInput")
    ident_d = nc.dram_tensor("ident_d", [P, P], FP, kind="ExternalInput")
    if mask_mode == "causal":
        mask_d = nc.dram_tensor("mask_t", [nsb, sbt, P, SB], FP, kind="ExternalInput")
    elif mask_mode == "full":
        mask_d = nc.dram_tensor("mask_t", [nskt, nsb, P, SB], FP, kind="ExternalInput")
    else:
        mask_d = None
    ln1_d = nc.dram_tensor("ln1_wb", [P, 2 * nkt], FP, kind="ExternalInput") if ln1_affine else None
    ln2_d = nc.dram_tensor("ln2_wb", [P, 2 * nkt], FP, kind="ExternalInput") if ln2_affine else None

    kt_dram = nc.dram_tensor("ktd", [qo, P, seq], FP)
    v_dram = nc.dram_tensor("vd", [nskt, P, dvc], FP)
    ar_in = nc.dram_tensor("ar_in", [nsb, hid, SB], FP)
    ar_out = nc.dram_tensor("ar_out", [nsb, hid, SB], FP)
    x2t_d = nc.dram_tensor("x2t", [hid, seq], FP)
    y_d = nc.dram_tensor("y_t", [hid, seq], FP, kind="ExternalOutput")

    q_nope_scale = 1.0 / math.sqrt(d_nope)
    q_rope_scale = 1.0 / math.sqrt(d_rope)

    _lp = nc.allow_low_precision(
        reason="float32r SBUF views are bit-identical fp32; PE truncates on read")
    _lp.__enter__()
    with tile.TileContext(nc) as tc:
        with (
            tc.tile_pool(name="const", bufs=1) as constp,
            tc.tile_pool(name="stat2", bufs=1) as stat2p,
            tc.tile_pool(name="psmm", bufs=6, space="PSUM") as psmm,
        ):
            ones_col = constp.tile([P, 1], FP)
            nc.sync.dma_start(fr(ones_col[:]), fr(ones_d[:, 0:1]))
            ones_row = constp.tile([1, P], FP)
            nc.sync.dma_start(fr(ones_row[:]), fr(ones_d[0:1, :]))
            eps_t = constp.tile([1, 1], FP)
            nc.any.memset(eps_t[:], EPS)
            rperm_t = constp.tile([P, P], FP)
            nc.sync.dma_start(fr(rperm_t[:]), fr(rp_d[:, :]))
            if mask_d is not None:
                ident = constp.tile([P, P], FP)
                nc.sync.dma_start(fr(ident[:]), fr(ident_d[:, :]))
            if ln1_affine:
                ln1_wb = constp.tile([P, 2 * nkt], FP)
                nc.sync.dma_start(ln1_wb[:], ln1_d[:, :])
            if ln2_affine:
                ln2_wb = constp.tile([P, 2 * nkt], FP)
                nc.sync.dma_start(ln2_wb[:], ln2_d[:, :])
            a2_all = stat2p.tile([1, seq], FP, tag="a2")
            c2_all = stat2p.tile([1, seq], FP, tag="c2")

            def bcast(row_sbuf):
                """[1, n<=SB] sbuf -> [P, n] psum via PE rank-1 matmul."""
                n = row_sbuf.shape[-1]
                ps = psmm.tile([P, SB], FP, tag="mm", name="bc")
                ps = ps[:, :n]
                nc.tensor.matmul(ps, fr(ones_row[:]), fr(row_sbuf), start=True, stop=True)
                return ps

            # ---------------- attention block ----------------
            with (
                tc.tile_pool(name="ht", bufs=1) as htp,
                tc.tile_pool(name="stats", bufs=2) as statp,
                tc.tile_pool(name="qt", bufs=1) as qtp,
                tc.tile_pool(name="expp", bufs=3) as expp,
                tc.tile_pool(name="ctxp", bufs=1) as ctxp,
                tc.tile_pool(name="wstr", bufs=4) as wsp,
                tc.tile_pool(name="trig", bufs=1) as trigp,
                tc.tile_pool(name="sqp", bufs=2) as sqp,
                tc.tile_pool(name="maskp", bufs=2) as mp,
                tc.tile_pool(name="miscp", bufs=2) as miscp,
                tc.tile_pool(name="psln", bufs=1, space="PSUM") as psln,
            ):
                def ln_stats(sum_ps, ssq_ps, sb_sl):
                    mu = statp.tile([1, SB], FP, tag="mu")
                    nc.scalar.activation(mu[:], sum_ps[:1, :], AF.Copy, scale=1.0 / hid)
                    msq = statp.tile([1, SB], FP, tag="msq")
                    nc.scalar.activation(msq[:], ssq_ps[:1, :], AF.Copy, scale=1.0 / hid)
                    var = statp.tile([1, SB], FP, tag="var")
                    nc.vector.tensor_tensor(var[:], mu[:], mu[:], ALU.mult)
                    nc.vector.tensor_tensor(var[:], msq[:], var[:], ALU.subtract)
                    std = statp.tile([1, SB], FP, tag="std")
                    nc.scalar.activation(std[:], var[:], AF.Sqrt, bias=eps_t[:])
                    rstd = statp.tile([1, SB], FP, tag="rstd")
                    nc.vector.reciprocal(fr(rstd[:]), std[:])
                    nmr = statp.tile([1, SB], FP, tag="nmr")
                    nc.vector.tensor_tensor(fr(nmr[:]), mu[:], rstd[:], ALU.mult)
                    nc.vector.tensor_scalar_mul(fr(nmr[:]), nmr[:], -1.0)
                    return rstd, nmr

                def rope_apply(dest, raw, cs, sn):
                    """dest/raw: [P, SB]; rows (per 64-pair): x1 | x2.
                    out = raw*cos + swap(raw)*sin_signed, swap via PE perm."""
                    ps_sw = psmm.tile([P, SB], FP, tag="mm", name="swp")
                    nc.tensor.matmul(ps_sw[:], fr(rperm_t[:]), fr(raw[:]),
                                     start=True, stop=True)
                    m1 = miscp.tile([P, SB], FP, tag="m1")
                    m2 = miscp.tile([P, SB], FP, tag="m2")
                    nc.vector.tensor_tensor(m1[:], raw[:], cs, ALU.mult)
                    nc.vector.tensor_tensor(m2[:], ps_sw[:], sn, ALU.mult)
                    nc.vector.tensor_tensor(fr(dest), m1[:], m2[:], ALU.add)

                for sb in range(nsb):
                    ssl = slice(sb * SB, (sb + 1) * SB)
                    # --- stage A: LN1 into ht (in place over the x tiles) ---
                    ht = htp.tile([P, nkt, SB], FP, tag="ht")
                    sum_ps = psln.tile([1, SB], FP, tag="lsum")
                    ssq_ps = psln.tile([1, SB], FP, tag="lssq")
                    for kt in range(nkt):
                        nc.sync.dma_start(fr(ht[:, kt, :]), fr(xt_d[kt * P:(kt + 1) * P, ssl]))
                        sq = sqp.tile([P, SB], FP, tag="sq")
                        nc.vector.tensor_tensor(fr(sq[:]), ht[:, kt, :], ht[:, kt, :], ALU.mult)
                        nc.tensor.matmul(sum_ps[:], fr(ones_col[:]), fr(ht[:, kt, :]),
                                         start=(kt == 0), stop=(kt == nkt - 1))
                        nc.tensor.matmul(ssq_ps[:], fr(ones_col[:]), fr(sq[:]),
                                         start=(kt == 0), stop=(kt == nkt - 1))
                    rstd, nmr = ln_stats(sum_ps, ssq_ps, ssl)
                    ab_ps = bcast(rstd[:])
                    cb_ps = bcast(nmr[:])
                    for kt in range(nkt):
                        nc.vector.tensor_tensor(fr(ht[:, kt, :]), ht[:, kt, :], ab_ps, ALU.mult)
                        nc.vector.tensor_tensor(fr(ht[:, kt, :]), ht[:, kt, :], cb_ps, ALU.add)
                        if ln1_affine:
                            nc.vector.tensor_scalar(
                                fr(ht[:, kt, :]), ht[:, kt, :],
                                ln1_wb[:, kt:kt + 1], ln1_wb[:, nkt + kt:nkt + kt + 1],
                                ALU.mult, ALU.add)

                    # --- stage B: q/k/v projections for this s-block ---
                    cs_t = trigp.tile([P, SB], FP, tag="cos")
                    sn_t = trigp.tile([P, SB], FP, tag="sin")
                    nc.sync.dma_start(cs_t[:], cos_d[:, ssl])
                    nc.sync.dma_start(sn_t[:], sin_d[:, ssl])
                    qt = qtp.tile([P, qo, SB], FP, tag="qt")
                    for which, w_d in (("q", wq_d), ("k", wk_d)):
                        for ot in range(qo):
                            mm_ps = psmm.tile([P, SB], FP, tag="mm")
                            for kt in range(nkt):
                                wch = wsp.tile([P, P], FP, tag="w")
                                nc.sync.dma_start(fr(wch[:]), fr(w_d[kt, ot]))
                                nc.tensor.matmul(mm_ps[:], fr(wch[:]), fr(ht[:, kt, :]),
                                                 start=(kt == 0), stop=(kt == nkt - 1))
                            is_rope = ot >= qo - n_rope_ot
                            if which == "q":
                                scale = q_rope_scale if is_rope else q_nope_scale
                                dest = qt[:, ot, :]
                            else:
                                scale = 1.0
                                stg = miscp.tile([P, SB], FP, tag="kvst")
                                dest = stg[:]
                            if not is_rope:
                                nc.scalar.activation(fr(dest), mm_ps[:], AF.Copy, scale=scale)
                            else:
                                raw = miscp.tile([P, SB], FP, tag="raw")
                                nc.scalar.activation(fr(raw[:]), mm_ps[:], AF.Copy, scale=scale)
                                rope_apply(dest, raw, cs_t[:], sn_t[:])
                            if which == "k":
                                nc.sync.dma_start(kt_dram[ot, :, ssl], stg[:])
                    v_pss = [psmm.tile([P, dvc], FP, tag="mm", name=f"vps{_i}") for _i in range(sbt)]
                    for kt in range(nkt):
                        wvch = wsp.tile([P, dvc], FP, tag="wv")
                        nc.sync.dma_start(fr(wvch[:]), fr(wv_d[kt]))
                        for sc in range(sbt):
                            nc.tensor.matmul(
                                v_pss[sc][:], fr(ht[:, kt, sc * P:(sc + 1) * P]), fr(wvch[:]),
                                start=(kt == 0), stop=(kt == nkt - 1))
                    for sc in range(sbt):
                        vst = miscp.tile([P, dvc], FP, tag="kvst")
                        nc.vector.tensor_copy(out=vst[:], in_=v_pss[sc][:])
                        nc.sync.dma_start(v_dram[sb * sbt + sc], vst[:])

                    # --- stage C: attention for q-block sb ---
                    t_max = (sb + 1) * sbt if mask_mode == "causal" else nskt
                    ctxt = ctxp.tile([P, hpc, SB], FP, tag="ctx")
                    for h in range(hpc):
                        rot = qo - n_rope_ot + h // 2
                        rsl = slice(64 * (h % 2), 64 * (h % 2) + 64)
                        sum_ps = psmm.tile([1, SB], FP, tag="mm")
                        ctx_ps = psmm.tile([P, SB], FP, tag="mm")
                        for t in range(t_max):
                            st_ps = psmm.tile([P, SB], FP, tag="mm")
                            tsl = slice(t * P, (t + 1) * P)
                            has_mask = mask_d is not None and (
                                mask_mode == "full" or t >= sb * sbt)
                            kn = wsp.tile([P, P], FP, tag="kl")
                            nc.sync.dma_start(fr(kn[:]), fr(kt_dram[h, :, tsl]))
                            kr = wsp.tile([P, P], FP, tag="krl")
                            nc.sync.dma_start(fr(kr[:]), fr(kt_dram[rot, :, tsl]))
                            vl = wsp.tile([P, P], FP, tag="vl")
                            nc.sync.dma_start(fr(vl[:]), fr(v_dram[t, :, h * P:(h + 1) * P]))
                            nc.tensor.matmul(st_ps[:], fr(kn[:]),
                                             fr(qt[:, h, :]), start=True, stop=False)
                            nc.tensor.matmul(st_ps[:], fr(kr[rsl, :]),
                                             fr(qt[rsl, rot, :]),
                                             start=False, stop=not has_mask)
                            if has_mask:
                                mt = mp.tile([P, SB], FP, tag="mask")
                                if mask_mode == "causal":
                                    nc.sync.dma_start(fr(mt[:]), fr(mask_d[sb, t - sb * sbt]))
                                else:
                                    nc.sync.dma_start(fr(mt[:]), fr(mask_d[t, sb]))
                                nc.tensor.matmul(st_ps[:], fr(ident[:]), fr(mt[:]),
                                                 start=False, stop=True)
                            es = expp.tile([P, SB], FP, tag="es")
                            nc.scalar.activation(fr(es[:]), st_ps[:], AF.Exp)
                            nc.tensor.matmul(sum_ps[:], fr(ones_col[:]), fr(es[:]),
                                             start=(t == 0), stop=(t == t_max - 1))
                            nc.tensor.matmul(ctx_ps[:], fr(vl[:]),
                                             fr(es[:]), start=(t == 0), stop=(t == t_max - 1))
                        rec = statp.tile([1, SB], FP, tag="rec")
                        nc.vector.reciprocal(fr(rec[:]), sum_ps[:1, :])
                        rb_ps = bcast(rec[:])
                        rb = miscp.tile([P, SB], FP, tag="rb")
                        nc.scalar.activation(rb[:], rb_ps[:], AF.Copy)
                        nc.vector.tensor_tensor(fr(ctxt[:, h, :]), ctx_ps[:], rb[:], ALU.mult)

                    # --- stage D: partial o_proj -> ar_in ---
                    for hc in range(nkt):
                        o_ps = psmm.tile([P, SB], FP, tag="mm")
                        for dvt in range(ndvt):
                            wch = wsp.tile([P, P], FP, tag="w")
                            nc.sync.dma_start(fr(wch[:]), fr(wo_d[dvt, hc]))
                            nc.tensor.matmul(o_ps[:], fr(wch[:]), fr(ctxt[:, dvt, :]),
                                             start=(dvt == 0), stop=(dvt == ndvt - 1))
                        ao = miscp.tile([P, SB], FP, tag="m1")
                        nc.vector.tensor_copy(out=ao[:], in_=o_ps[:])
                        nc.sync.dma_start(ar_in[sb, hc * P:(hc + 1) * P, :], ao[:])
                    nc.gpsimd.collective_compute(
                        "AllReduce", ALU.add,
                        replica_groups=[list(range(n_cores))],
                        ins=[ar_in[sb].opt()], outs=[ar_out[sb].opt()])

                # --- stage E: x2 = x + attn_out; LN2 stats; x2t to DRAM ---
                for sb in range(nsb):
                    ssl = slice(sb * SB, (sb + 1) * SB)
                    sum_ps = psln.tile([1, SB], FP, tag="lsum")
                    ssq_ps = psln.tile([1, SB], FP, tag="lssq")
                    for kt in range(nkt):
                        xtile = miscp.tile([P, SB], FP, tag="m2")
                        nc.sync.dma_start(fr(xtile[:]), fr(xt_d[kt * P:(kt + 1) * P, ssl]))
                        artile = miscp.tile([P, SB], FP, tag="raw")
                        nc.sync.dma_start(artile[:], ar_out[sb, kt * P:(kt + 1) * P, :])
                        nc.vector.tensor_tensor(fr(xtile[:]), xtile[:], artile[:], ALU.add)
                        nc.sync.dma_start(x2t_d[kt * P:(kt + 1) * P, ssl], xtile[:])
                        sq = sqp.tile([P, SB], FP, tag="sq")
                        nc.vector.tensor_tensor(fr(sq[:]), xtile[:], xtile[:], ALU.mult)
                        nc.tensor.matmul(sum_ps[:], fr(ones_col[:]), fr(xtile[:]),
                                         start=(kt == 0), stop=(kt == nkt - 1))
                        nc.tensor.matmul(ssq_ps[:], fr(ones_col[:]), fr(sq[:]),
                                         start=(kt == 0), stop=(kt == nkt - 1))
                    rstd, nmr = ln_stats(sum_ps, ssq_ps, ssl)
                    nc.vector.tensor_copy(out=fr(a2_all[:, ssl]), in_=rstd[:])
                    nc.vector.tensor_copy(out=fr(c2_all[:, ssl]), in_=nmr[:])

            # ---------------- FFN block ----------------
            with (
                tc.tile_pool(name="h2p", bufs=1) as h2p,
                tc.tile_pool(name="utp", bufs=1) as utp,
                tc.tile_pool(name="wfp", bufs=4) as wfp,
                tc.tile_pool(name="x2sp", bufs=2) as x2sp,
            ):
                for fb in range(nfb):
                    fsl = slice(fb * FB, (fb + 1) * FB)
                    h2 = h2p.tile([P, nkt, FB], FP, tag="h2")
                    ab_pss, cb_pss = [], []
                    for sub in range(nsub):
                        st = slice(fb * FB + sub * SB, fb * FB + (sub + 1) * SB)
                        ab_pss.append(bcast(a2_all[:, st]))
                        cb_pss.append(bcast(c2_all[:, st]))
                    for kt in range(nkt):
                        for sub in range(nsub):
                            dsl = slice(sub * SB, (sub + 1) * SB)
                            st = slice(fb * FB + sub * SB, fb * FB + (sub + 1) * SB)
                            x2tile = x2sp.tile([P, SB], FP, tag="x2l")
                            nc.sync.dma_start(x2tile[:], x2t_d[kt * P:(kt + 1) * P, st])
                            nc.vector.tensor_tensor(fr(h2[:, kt, dsl]), x2tile[:], ab_pss[sub], ALU.mult)
                            nc.vector.tensor_tensor(fr(h2[:, kt, dsl]), h2[:, kt, dsl], cb_pss[sub], ALU.add)
                            if ln2_affine:
                                nc.vector.tensor_scalar(
                                    fr(h2[:, kt, dsl]), h2[:, kt, dsl],
                                    ln2_wb[:, kt:kt + 1], ln2_wb[:, nkt + kt:nkt + kt + 1],
                                    ALU.mult, ALU.add)
                    ut = utp.tile([P, nft, FB], BF, tag="ut")
                    for ft in range(nft):
                        u_pss = [psmm.tile([P, SB], FP, tag="mm", name=f"ups{_i}") for _i in range(nsub)]
                        for kt in range(nkt):
                            wch = wfp.tile([P, P], FP, tag="w1")
                            nc.sync.dma_start(fr(wch[:]), fr(w1_d[kt, ft]))
                            for sub in range(nsub):
                                nc.tensor.matmul(
                                    u_pss[sub][:], fr(wch[:]),
                                    fr(h2[:, kt, sub * SB:(sub + 1) * SB]),
                                    start=(kt == 0), stop=(kt == nkt - 1))
                        for sub in range(nsub):
                            nc.scalar.activation(ut[:, ft, sub * SB:(sub + 1) * SB],
                                                 u_pss[sub][:], AF.Silu)
                    for hc in range(nkt):
                        y_pss = [psmm.tile([P, SB], FP, tag="mm", name=f"yps{_i}") for _i in range(nsub)]
                        for ft in range(nft):
                            wch = wfp.tile([P, P], BF, tag="w2")
                            nc.sync.dma_start(wch[:], w2_d[ft, hc])
                            for sub in range(nsub):
                                nc.tensor.matmul(
                                    y_pss[sub][:], wch[:],
                                    ut[:, ft, sub * SB:(sub + 1) * SB],
                                    start=(ft == 0), stop=(ft == nft - 1))
                        for sub in range(nsub):
                            st = slice(fb * FB + sub * SB, fb * FB + (sub + 1) * SB)
                            x2tile = x2sp.tile([P, SB], FP, tag="x2r")
                            nc.sync.dma_start(x2tile[:], x2t_d[hc * P:(hc + 1) * P, st])
                            yt = x2sp.tile([P, SB], FP, tag="yt")
                            nc.vector.tensor_scalar_mul(yt[:], x2tile[:], 1.0 / n_cores)
                            nc.vector.tensor_tensor(yt[:], y_pss[sub][:], yt[:], ALU.add)
                            nc.sync.dma_start(y_d[hc * P:(hc + 1) * P, st], yt[:])

    _lp.__exit__(None, None, None)
    nc.compile()
    return nc


# ---------------------------------------------------------------------------
# host side
# ---------------------------------------------------------------------------

def _chunk2d(a, pr, pc):
    """[R, C] -> [R//pr, C//pc, pr, pc] contiguous chunk layout."""
    R, C = a.shape
    return np.ascontiguousarray(
        a.reshape(R // pr, pr, C // pc, pc).transpose(0, 2, 1, 3))


def make_core_inputs(inputs, cfg, mask_mode, ln1_affine, ln2_affine):
    seq, hid, ffn = cfg["seq"], cfg["hid"], cfg["ffn"]
    n_cores, n_heads = cfg["n_cores"], cfg["n_heads"]
    d_nope, d_rope, d_v = cfg["d_nope"], cfg["d_rope"], cfg["d_v"]
    SB = cfg["sb"]
    hpc = n_heads // n_cores
    nkt = hid // P
    nsb = seq // SB
    sbt = SB // P
    nskt = seq // P
    fpc = ffn // n_cores

    f32 = np.float32
    x = np.asarray(inputs["hidden_states"], dtype=f32)[0]        # [seq, hid]
    xt = np.ascontiguousarray(x.T)                                # [hid, seq]

    inv = (1.0 / (10000.0 ** (np.arange(0, d_rope, 2, dtype=f32) / f32(d_rope)))).astype(f32)
    t = np.arange(seq, dtype=f32)
    freqs = t[:, None] * inv[None, :]
    cosT = np.cos(freqs).astype(f32).T                      # [half, seq]
    sinT = np.sin(freqs).astype(f32).T
    cos128 = np.ascontiguousarray(np.tile(cosT, (P // (d_rope // 2), 1)))
    sin128 = np.ascontiguousarray(
        np.tile(np.concatenate([-sinT, sinT], axis=0), (P // d_rope, 1)))
    half = d_rope // 2
    rperm = np.zeros((P, P), dtype=f32)
    for blk in range(P // d_rope):
        b = blk * d_rope
        for i in range(half):
            # out[b+i] takes in[b+half+i]; out[b+half+i] takes in[b+i]
            rperm[b + half + i, b + i] = 1.0
            rperm[b + i, b + half + i] = 1.0

    common = {"xt": xt, "cos_t": cos128, "sin_t": sin128, "rperm": rperm,
              "ones_d": np.ones((P, P), dtype=f32),
              "ident_d": np.eye(P, dtype=f32)}
    mask = np.asarray(inputs["attention_mask"], dtype=f32)[0, 0]  # [seq, seq]
    mT = np.ascontiguousarray(mask.T)                             # [sk, sq]
    if mask_mode == "causal":
        m = np.empty((nsb, sbt, P, SB), dtype=f32)
        for qb in range(nsb):
            for i in range(sbt):
                tt = qb * sbt + i
                m[qb, i] = mT[tt * P:(tt + 1) * P, qb * SB:(qb + 1) * SB]
        common["mask_t"] = m
    elif mask_mode == "full":
        m = np.empty((nskt, nsb, P, SB), dtype=f32)
        for tt in range(nskt):
            for qb in range(nsb):
                m[tt, qb] = mT[tt * P:(tt + 1) * P, qb * SB:(qb + 1) * SB]
        common["mask_t"] = m
    if ln1_affine:
        common["ln1_wb"] = np.ascontiguousarray(np.stack(
            [np.asarray(inputs["ln1_w"], f32), np.asarray(inputs["ln1_b"], f32)]
        ).reshape(2, nkt, P).transpose(2, 0, 1).reshape(P, 2 * nkt))
    if ln2_affine:
        common["ln2_wb"] = np.ascontiguousarray(np.stack(
            [np.asarray(inputs["ln2_w"], f32), np.asarray(inputs["ln2_b"], f32)]
        ).reshape(2, nkt, P).transpose(2, 0, 1).reshape(P, 2 * nkt))

    wq = np.asarray(inputs["w_q"], f32)
    wk = np.asarray(inputs["w_k"], f32)
    wv = np.asarray(inputs["w_v"], f32)
    wo = np.asarray(inputs["w_o"], f32)
    w1 = np.asarray(inputs["w1"], f32)
    w2 = np.asarray(inputs["w2"], f32)

    in_maps = []
    for c in range(n_cores):
        heads = range(c * hpc, (c + 1) * hpc)
        nope = np.concatenate([wq[g * d_nope:(g + 1) * d_nope] for g in heads])
        rope = np.concatenate(
            [wq[n_heads * d_nope + g * d_rope: n_heads * d_nope + (g + 1) * d_rope]
             for g in heads])
        wq_t = _chunk2d(np.concatenate([nope, rope]).T, P, P)
        nope = np.concatenate([wk[g * d_nope:(g + 1) * d_nope] for g in heads])
        rope = np.concatenate(
            [wk[n_heads * d_nope + g * d_rope: n_heads * d_nope + (g + 1) * d_rope]
             for g in heads])
        wk_t = _chunk2d(np.concatenate([nope, rope]).T, P, P)
        wv_c = np.concatenate([wv[g * d_v:(g + 1) * d_v] for g in heads])   # [dvc, hid]
        wv_t = np.ascontiguousarray(wv_c.T.reshape(nkt, P, hpc * d_v))
        wo_c = wo[:, c * hpc * d_v:(c + 1) * hpc * d_v]                      # [hid, dvc]
        wo_t = _chunk2d(np.ascontiguousarray(wo_c.T), P, P)
        w1_t = _chunk2d(np.ascontiguousarray(w1[c * fpc:(c + 1) * fpc].T), P, P)
        w2_t = _chunk2d(np.ascontiguousarray(w2[:, c * fpc:(c + 1) * fpc].T), P, P).astype(ml_dtypes.bfloat16)
        in_maps.append(dict(common, wq_t=wq_t, wk_t=wk_t, wv_t=wv_t, wo_t=wo_t,
                            w1_t=w1_t, w2_t=w2_t))
    return in_maps


def detect_mask_mode(mask, seq):
    if not mask.any():
        return "zero"
    iu = np.triu_indices(seq, 1)
    upper_blocked = bool((mask[iu] <= -1e8).all())
    il = np.tril_indices(seq)
    lower_zero = bool((mask[il] == 0).all())
    if upper_blocked and lower_zero:
        return "causal"
    return "full"


_BUILT = {}


def run_layer(inputs, cfg, trace=False):
    f32 = np.float32
    mask = np.asarray(inputs["attention_mask"], dtype=f32)[0, 0]
    mask_mode = detect_mask_mode(mask, cfg["seq"])
    ln1_affine = not ((np.asarray(inputs["ln1_w"]) == 1).all()
                     and (np.asarray(inputs["ln1_b"]) == 0).all())
    ln2_affine = not ((np.asarray(inputs["ln2_w"]) == 1).all()
                     and (np.asarray(inputs["ln2_b"]) == 0).all())
    key = (tuple(sorted(cfg.items())), mask_mode, ln1_affine, ln2_affine)
    if key not in _BUILT:
        _BUILT[key] = build_layer_kernel(cfg, mask_mode, ln1_affine, ln2_affine)
    nc = _BUILT[key]
    in_maps = make_core_inputs(inputs, cfg, mask_mode, ln1_affine, ln2_affine)
    res = run_bass_kernel_spmd(nc, in_maps, core_ids=list(range(cfg["n_cores"])),
                               trace=trace)
    acc = np.zeros((cfg["hid"], cfg["seq"]), dtype=np.float64)
    for c in range(cfg["n_cores"]):
        acc += res.results[c]["y_t"]
    out = acc.T.astype(f32)[None]
    return out, res


def kernel(**inputs):
    out, _ = run_layer(inputs, CFG_FULL)
    return out



# revision 13
# speedup vs baseline: 1.8817x; 1.8817x over previous
"""Fused tensor-parallel transformer layer for Trainium2 (8 NeuronCores).

Sharding: Megatron-style tensor parallel. Each core owns 4 heads of the
attention block (q/k/v projection rows, o_proj columns) and 1/8 of the FFN
hidden dim (w1 rows, w2 columns). LayerNorms are computed replicated on
every core. One on-device AllReduce (bf16, Shared output) joins the
attention block to the FFN block; the final residual sum is assembled on
the host from per-core partial outputs (each core adds x2/8 so the
partials sum to the answer).

v2 layout: all matmul operands are bf16 (fast weight load + half the HBM
traffic), weights are DMA'd in one batched transfer per output tile,
K/V stay resident in SBUF (no DRAM round trip), the causal mask is a 0/1
multiply on the vector engine, and DMA issue is spread across the sync /
scalar / gpsimd queues. Activations stay transposed ([hid, seq]) so every
matmul contracts over the partition dim with zero on-device transposes.
"""

import math
import ml_dtypes
import numpy as np

import concourse.bass as bass
import concourse.mybir as mybir
import concourse.tile as tile
from concourse import bacc
from concourse.bass_utils import run_bass_kernel_spmd

FP = mybir.dt.float32
BF = mybir.dt.bfloat16
P = 128
EPS = 1e-6
AF = mybir.ActivationFunctionType
ALU = mybir.AluOpType
BF_NP = ml_dtypes.bfloat16


def fr(ap):
    return ap.bitcast(mybir.dt.float32r)


CFG_FULL = dict(
    seq=2048, hid=4096, ffn=16384, n_cores=8, n_heads=32,
    d_nope=128, d_rope=64, d_v=128, sb=512, ss=1024, fb=1024,
)


def build_layer_kernel(cfg, mask_mode, ln1_affine, ln2_affine):
    """mask_mode: 'causal' (skip tiles above diag, 0/1-multiply diag tiles),
    'zero' (no mask at all), 'full' (additive mask everywhere)."""
    seq, hid, ffn = cfg["seq"], cfg["hid"], cfg["ffn"]
    n_cores, n_heads = cfg["n_cores"], cfg["n_heads"]
    d_nope, d_rope, d_v = cfg["d_nope"], cfg["d_rope"], cfg["d_v"]
    SB, SS, FB = cfg["sb"], cfg["ss"], cfg["fb"]
    half = d_rope // 2
    hpc = n_heads // n_cores              # heads per core
    nkt = hid // P                        # hid k-tiles
    nsb = seq // SB                       # 512-wide blocks (attn q / stage E)
    sbt = SB // P                         # sk tiles per 512 block
    nskt = seq // P                       # total sk tiles
    nss = seq // SS                       # projection super blocks
    psub = SS // SB                       # 512 sub blocks per super block
    n_rope_ot = hpc * d_rope // P         # rope o-tiles (2 heads each)
    qo = hpc * d_nope // P + n_rope_ot    # q/k o-tiles per core
    dvc = hpc * d_v                       # v cols per core
    ndvt = dvc // P                       # o_proj contraction tiles
    fpc = ffn // n_cores                  # ffn rows per core
    nft = fpc // P                        # f tiles per core
    nfb = seq // FB                       # ffn s-blocks
    fsub = FB // SB                       # 512 sub blocks per ffn block
    assert hpc % 2 == 0 and half == 32 and d_nope == P and d_v == P

    nc = bacc.Bacc(None, target_bir_lowering=False)

    xt_d = nc.dram_tensor("xt", [nkt, P, seq], BF, kind="ExternalInput")
    wqk_d = nc.dram_tensor("wqk_t", [2 * qo, P, nkt * P], BF, kind="ExternalInput")
    wv_d = nc.dram_tensor("wv_t", [nkt, P, dvc], BF, kind="ExternalInput")
    wo_d = nc.dram_tensor("wo_t", [nkt, P, ndvt * P], BF, kind="ExternalInput")
    w1_d = nc.dram_tensor("w1_t", [nft, P, nkt * P], BF, kind="ExternalInput")
    w2_d = nc.dram_tensor("w2_t", [nkt, P, nft * P], BF, kind="ExternalInput")
    cos_d = nc.dram_tensor("cos_t", [P, seq], BF, kind="ExternalInput")
    sin_d = nc.dram_tensor("sin_t", [P, seq], FP, kind="ExternalInput")
    rp_d = nc.dram_tensor("rperm", [P, P], BF, kind="ExternalInput")
    onc_d = nc.dram_tensor("onc", [P, 1], BF, kind="ExternalInput")
    onr_d = nc.dram_tensor("onr", [1, P], FP, kind="ExternalInput")
    if mask_mode == "causal":
        mask_d = nc.dram_tensor("mask_t", [nsb, P, sbt * SB], BF, kind="ExternalInput")
    elif mask_mode == "full":
        mask_d = nc.dram_tensor("mask_t", [nskt, nsb, P, SB], FP, kind="ExternalInput")
    else:
        mask_d = None
    ln1_d = nc.dram_tensor("ln1_wb", [P, 2 * nkt], FP, kind="ExternalInput") if ln1_affine else None
    ln2_d = nc.dram_tensor("ln2_wb", [P, 2 * nkt], FP, kind="ExternalInput") if ln2_affine else None

    ar_in = nc.dram_tensor("ar_in", [nsb, nkt, P, SB], BF)
    ar_out = nc.dram_tensor("ar_out", [nsb, nkt, P, SB], BF, addr_space="Shared")
    x2_d = nc.dram_tensor("x2t", [nsb, nkt, P, SB], BF)
    y_d = nc.dram_tensor("y_t", [nkt, P, seq], FP, kind="ExternalOutput")

    q_nope_scale = 1.0 / math.sqrt(d_nope)
    q_rope_scale = 1.0 / math.sqrt(d_rope)

    _lp = nc.allow_low_precision(
        reason="bf16 matmul operands; fp32 SBUF views bitcast to float32r")
    _lp.__enter__()
    with tile.TileContext(nc) as tc:
        with (
            tc.tile_pool(name="const", bufs=1) as constp,
            tc.tile_pool(name="stat2", bufs=1) as stat2p,
            tc.tile_pool(name="stats", bufs=1) as statp,
            tc.tile_pool(name="sqp", bufs=2) as sqp,
            tc.tile_pool(name="xep", bufs=2) as xep,
            tc.tile_pool(name="psmm", bufs=6, space="PSUM") as psmm,
            tc.tile_pool(name="psln", bufs=1, space="PSUM") as psln,
        ):
            ones_col = constp.tile([P, 1], BF)
            nc.sync.dma_start(ones_col[:], onc_d[:, :])
            ones_row = constp.tile([1, P], FP)
            nc.sync.dma_start(fr(ones_row[:]), fr(onr_d[:, :]))
            ones_row_bf = constp.tile([1, P], BF)
            nc.vector.tensor_copy(out=ones_row_bf[:], in_=ones_row[:])
            eps_t = constp.tile([1, 1], FP)
            nc.any.memset(eps_t[:], EPS)
            rperm_t = constp.tile([P, P], BF)
            nc.sync.dma_start(rperm_t[:], rp_d[:, :])
            if ln1_affine:
                ln1_wb = constp.tile([P, 2 * nkt], FP)
                nc.sync.dma_start(ln1_wb[:], ln1_d[:, :])
            if ln2_affine:
                ln2_wb = constp.tile([P, 2 * nkt], FP)
                nc.sync.dma_start(ln2_wb[:], ln2_d[:, :])
            a2_all = stat2p.tile([1, seq], BF, tag="a2")
            c2_all = stat2p.tile([1, seq], BF, tag="c2")

            def bcast(row_sbuf):
                """[1, n<=SB] sbuf row -> [P, n] psum via PE rank-1 matmul."""
                n = row_sbuf.shape[-1]
                ps = psmm.tile([P, SB], FP, tag="mm", name="bc")
                ps = ps[:, :n]
                if row_sbuf.dtype == BF:
                    nc.tensor.matmul(ps, ones_row_bf[:], row_sbuf, start=True, stop=True)
                else:
                    nc.tensor.matmul(ps, fr(ones_row[:]), fr(row_sbuf), start=True, stop=True)
                return ps

            def ln_stats(sum_ps, ssq_ps):
                mu = statp.tile([1, SB], FP, tag="mu")
                nc.scalar.activation(mu[:], sum_ps[:1, :], AF.Copy, scale=1.0 / hid)
                msq = statp.tile([1, SB], FP, tag="msq")
                nc.scalar.activation(msq[:], ssq_ps[:1, :], AF.Copy, scale=1.0 / hid)
                var = statp.tile([1, SB], FP, tag="var")
                nc.vector.tensor_tensor(var[:], mu[:], mu[:], ALU.mult)
                nc.vector.tensor_tensor(var[:], msq[:], var[:], ALU.subtract)
                std = statp.tile([1, SB], FP, tag="std")
                nc.scalar.activation(std[:], var[:], AF.Sqrt, bias=eps_t[:])
                rstd = statp.tile([1, SB], FP, tag="rstd")
                nc.vector.reciprocal(fr(rstd[:]), std[:])
                nmr = statp.tile([1, SB], FP, tag="nmr")
                nc.vector.tensor_tensor(fr(nmr[:]), mu[:], rstd[:], ALU.mult)
                nc.vector.tensor_scalar_mul(fr(nmr[:]), nmr[:], -1.0)
                return rstd, nmr

            def stage_e(sb):
                """x2 = x + attn_allreduce; write x2 (bf16); LN2 stats."""
                ssl = slice(sb * SB, (sb + 1) * SB)
                sum_ps = psln.tile([1, SB], FP, tag="lsum")
                ssq_ps = psln.tile([1, SB], FP, tag="lssq")
                for kt in range(nkt):
                    xe = xep.tile([P, SB], BF, tag="xe")
                    nc.scalar.dma_start(xe[:], xt_d[kt, :, ssl])
                    are = xep.tile([P, SB], BF, tag="are")
                    nc.scalar.dma_start(are[:], ar_out[sb, kt])
                    x2t = xep.tile([P, SB], BF, tag="x2w")
                    nc.vector.tensor_tensor(x2t[:], xe[:], are[:], ALU.add)
                    nc.scalar.dma_start(x2_d[sb, kt], x2t[:])
                    sq = sqp.tile([P, SB], BF, tag="sq")
                    nc.gpsimd.tensor_tensor(sq[:], x2t[:], x2t[:], ALU.mult)
                    nc.tensor.matmul(sum_ps[:], ones_col[:], x2t[:],
                                     start=(kt == 0), stop=(kt == nkt - 1))
                    nc.tensor.matmul(ssq_ps[:], ones_col[:], sq[:],
                                     start=(kt == 0), stop=(kt == nkt - 1))
                rstd, nmr = ln_stats(sum_ps, ssq_ps)
                nc.vector.tensor_copy(out=a2_all[:, ssl], in_=rstd[:])
                nc.vector.tensor_copy(out=c2_all[:, ssl], in_=nmr[:])

            # ---------------- attention block ----------------
            with (
                tc.tile_pool(name="ht", bufs=1) as htp,
                tc.tile_pool(name="qt", bufs=1) as qtp,
                tc.tile_pool(name="kall", bufs=1) as kallp,
                tc.tile_pool(name="vall", bufs=1) as vallp,
                tc.tile_pool(name="ctxp", bufs=1) as ctxp,
                tc.tile_pool(name="expp", bufs=3) as expp,
                tc.tile_pool(name="wqkp", bufs=2) as wqkp,
                tc.tile_pool(name="wvp", bufs=2) as wvp,
                tc.tile_pool(name="wop", bufs=2) as wop,
                tc.tile_pool(name="trig", bufs=1) as trigp,
                tc.tile_pool(name="maskp", bufs=1) as mp,
                tc.tile_pool(name="miscp", bufs=2) as miscp,
            ):
                k_all = kallp.tile([P, qo, seq], BF, tag="kall")
                v_all = vallp.tile([P, nskt, dvc], BF, tag="vall")

                def rope_apply(dest, raw, cs, sn):
                    """dest(bf16)/raw(bf16): [P, SB]; rows per 64-block: x1|x2.
                    out = raw*cos + swap(raw)*sin_signed, swap via PE perm."""
                    ps_sw = psmm.tile([P, SB], FP, tag="mm", name="swp")
                    nc.tensor.matmul(ps_sw[:], rperm_t[:], raw[:], start=True, stop=True)
                    m1 = miscp.tile([P, SB], FP, tag="mtmp", name="m1")
                    m2 = miscp.tile([P, SB], FP, tag="mtmp", name="m2")
                    nc.vector.tensor_tensor(m1[:], raw[:], cs, ALU.mult)
                    nc.vector.tensor_tensor(m2[:], ps_sw[:], sn, ALU.mult)
                    nc.vector.tensor_tensor(dest, m1[:], m2[:], ALU.add)

                def attn_superblock(ss):
                    ssl = slice(ss * SS, (ss + 1) * SS)
                    # --- stage A: load x tiles, LN1 stats, normalize in place
                    ht = htp.tile([P, nkt, SS], BF, tag="ht")
                    for kt in range(nkt):
                        nc.sync.dma_start(ht[:, kt, :], xt_d[kt, :, ssl])
                    for sub in range(psub):
                        dsl = slice(sub * SB, (sub + 1) * SB)
                        sum_ps = psln.tile([1, SB], FP, tag="lsum")
                        ssq_ps = psln.tile([1, SB], FP, tag="lssq")
                        for kt in range(nkt):
                            sq = sqp.tile([P, SB], BF, tag="sq")
                            nc.gpsimd.tensor_tensor(sq[:], ht[:, kt, dsl], ht[:, kt, dsl], ALU.mult)
                            nc.tensor.matmul(sum_ps[:], ones_col[:], ht[:, kt, dsl],
                                             start=(kt == 0), stop=(kt == nkt - 1))
                            nc.tensor.matmul(ssq_ps[:], ones_col[:], sq[:],
                                             start=(kt == 0), stop=(kt == nkt - 1))
                        rstd, nmr = ln_stats(sum_ps, ssq_ps)
                        ab_ps = bcast(rstd[:])
                        cb_ps = bcast(nmr[:])
                        for kt in range(nkt):
                            nc.vector.tensor_tensor(ht[:, kt, dsl], ht[:, kt, dsl], ab_ps, ALU.mult)
                            nc.vector.tensor_tensor(ht[:, kt, dsl], ht[:, kt, dsl], cb_ps, ALU.add)
                            if ln1_affine:
                                nc.vector.tensor_scalar(
                                    ht[:, kt, dsl], ht[:, kt, dsl],
                                    ln1_wb[:, kt:kt + 1], ln1_wb[:, nkt + kt:nkt + kt + 1],
                                    ALU.mult, ALU.add)

                    # --- stage B: q/k/v projections for this super block ---
                    cs_ss = trigp.tile([P, SS], BF, tag="cos")
                    nc.sync.dma_start(cs_ss[:], cos_d[:, ssl])
                    sn_ss = trigp.tile([P, SS], FP, tag="sin")
                    nc.sync.dma_start(fr(sn_ss[:]), fr(sin_d[:, ssl]))
                    qt = qtp.tile([P, qo, SS], BF, tag="qt")
                    for ot in range(2 * qo):
                        wt = wqkp.tile([P, nkt * P], BF, tag="wqk")
                        nc.sync.dma_start(wt[:], wqk_d[ot])
                        pss = [psmm.tile([P, SB], FP, tag="mm", name=f"proj{s}")
                               for s in range(psub)]
                        for kt in range(nkt):
                            for s2 in range(psub):
                                nc.tensor.matmul(
                                    pss[s2][:], wt[:, kt * P:(kt + 1) * P],
                                    ht[:, kt, s2 * SB:(s2 + 1) * SB],
                                    start=(kt == 0), stop=(kt == nkt - 1))
                        is_q = ot < qo
                        o = ot % qo
                        is_rope = o >= qo - n_rope_ot
                        for s2 in range(psub):
                            gsl = slice(ss * SS + s2 * SB, ss * SS + (s2 + 1) * SB)
                            if is_q:
                                dest = qt[:, o, s2 * SB:(s2 + 1) * SB]
                                scale = q_rope_scale if is_rope else q_nope_scale
                            else:
                                dest = k_all[:, o, gsl]
                                scale = 1.0
                            if not is_rope:
                                nc.scalar.activation(dest, pss[s2][:], AF.Copy, scale=scale)
                            else:
                                raw = miscp.tile([P, SB], BF, tag="raw")
                                nc.scalar.activation(raw[:], pss[s2][:], AF.Copy, scale=scale)
                                dsl2 = slice(s2 * SB, (s2 + 1) * SB)
                                rope_apply(dest, raw[:], cs_ss[:, dsl2], sn_ss[:, dsl2])

                    for hf in range(2):
                        v_pss = [psmm.tile([P, dvc], FP, tag="mm", name=f"vps{i}")
                                 for i in range(4)]
                        for kt in range(nkt):
                            wvt = wvp.tile([P, dvc], BF, tag="wv")
                            nc.sync.dma_start(wvt[:], wv_d[kt])
                            for i in range(4):
                                sc = hf * 4 + i
                                nc.tensor.matmul(
                                    v_pss[i][:], ht[:, kt, sc * P:(sc + 1) * P], wvt[:],
                                    start=(kt == 0), stop=(kt == nkt - 1))
                        for i in range(4):
                            t_idx = ss * (SS // P) + hf * 4 + i
                            nc.scalar.activation(v_all[:, t_idx, :], v_pss[i][:], AF.Copy)

                    # --- stages C/D/E per 512-wide q-block ---
                    for qb in range(psub):
                        sb = ss * psub + qb
                        qsl = slice(qb * SB, (qb + 1) * SB)
                        t_max = (sb + 1) * sbt if mask_mode == "causal" else nskt
                        if mask_mode == "causal":
                            mt = mp.tile([P, sbt * SB], BF, tag="mask")
                            nc.sync.dma_start(mt[:], mask_d[sb])
                        ctxt = ctxp.tile([P, hpc, SB], BF, tag="ctx")
                        for h in range(hpc):
                            rot = qo - n_rope_ot + h // 2
                            rsl = slice(64 * (h % 2), 64 * (h % 2) + 64)
                            sum_ps = psmm.tile([1, SB], FP, tag="mm", name="smx")
                            ctx_ps = psmm.tile([P, SB], FP, tag="mm", name="ctxps")
                            for t in range(t_max):
                                tsl = slice(t * P, (t + 1) * P)
                                st_ps = psmm.tile([P, SB], FP, tag="mm", name="st")
                                nc.tensor.matmul(st_ps[:], k_all[:, h, tsl],
                                                 qt[:, h, qsl], start=True, stop=False)
                                nc.tensor.matmul(st_ps[:], k_all[rsl, rot, tsl],
                                                 qt[rsl, rot, qsl],
                                                 start=False, stop=True)
                                es = expp.tile([P, SB], BF, tag="es")
                                if mask_mode == "full":
                                    mtf = mp.tile([P, SB], FP, tag="maskf")
                                    nc.sync.dma_start(fr(mtf[:]), fr(mask_d[t, sb]))
                                    stf = miscp.tile([P, SB], FP, tag="stf")
                                    nc.vector.tensor_tensor(stf[:], st_ps[:], mtf[:], ALU.add)
                                    nc.scalar.activation(es[:], stf[:], AF.Exp)
                                else:
                                    nc.scalar.activation(es[:], st_ps[:], AF.Exp)
                                if mask_mode == "causal" and t >= sb * sbt:
                                    i = t - sb * sbt
                                    nc.vector.tensor_tensor(
                                        es[:], es[:], mt[:, i * SB:(i + 1) * SB], ALU.mult)
                                nc.tensor.matmul(sum_ps[:], ones_col[:], es[:],
                                                 start=(t == 0), stop=(t == t_max - 1))
                                nc.tensor.matmul(ctx_ps[:], v_all[:, t, h * P:(h + 1) * P],
                                                 es[:], start=(t == 0), stop=(t == t_max - 1))
                            rec = statp.tile([1, SB], FP, tag="rec")
                            nc.vector.reciprocal(fr(rec[:]), sum_ps[:1, :])
                            rb_ps = bcast(rec[:])
                            rb = miscp.tile([P, SB], FP, tag="rb")
                            nc.scalar.activation(rb[:], rb_ps[:], AF.Copy)
                            nc.vector.tensor_tensor(ctxt[:, h, :], ctx_ps[:], rb[:], ALU.mult)

                        # --- stage D: partial o_proj -> ar_in ---
                        for hc in range(nkt):
                            wot = wop.tile([P, ndvt * P], BF, tag="wo")
                            nc.sync.dma_start(wot[:], wo_d[hc])
                            o_ps = psmm.tile([P, SB], FP, tag="mm", name="ops")
                            for dvt in range(ndvt):
                                nc.tensor.matmul(o_ps[:], wot[:, dvt * P:(dvt + 1) * P],
                                                 ctxt[:, dvt, :],
                                                 start=(dvt == 0), stop=(dvt == ndvt - 1))
                            ao = miscp.tile([P, SB], BF, tag="ao")
                            nc.scalar.activation(ao[:], o_ps[:], AF.Copy)
                            nc.gpsimd.dma_start(ar_in[sb, hc], ao[:])
                        nc.gpsimd.collective_compute(
                            "AllReduce", ALU.add,
                            replica_groups=[list(range(n_cores))],
                            ins=[ar_in[sb].opt()], outs=[ar_out[sb].opt()])
                        if sb >= 1:
                            stage_e(sb - 1)

                for ss in range(nss):
                    attn_superblock(ss)

            # ---------------- FFN block ----------------
            with (
                tc.tile_pool(name="h2p", bufs=1) as h2p,
                tc.tile_pool(name="utp", bufs=1) as utp,
                tc.tile_pool(name="w1p", bufs=2) as w1p,
                tc.tile_pool(name="w2p", bufs=2) as w2p,
                tc.tile_pool(name="x2sp", bufs=3) as x2sp,
                tc.tile_pool(name="ysp", bufs=2) as ysp,
            ):
                def ffn_block(fb):
                    if fb == nfb - 1:
                        stage_e(nsb - 1)
                    ab_pss, cb_pss = [], []
                    for sub in range(fsub):
                        gsl = slice(fb * FB + sub * SB, fb * FB + (sub + 1) * SB)
                        ab_pss.append(bcast(a2_all[:, gsl]))
                        cb_pss.append(bcast(c2_all[:, gsl]))
                    h2 = h2p.tile([P, nkt, FB], BF, tag="h2")
                    for kt in range(nkt):
                        for sub in range(fsub):
                            dsl = slice(sub * SB, (sub + 1) * SB)
                            x2t = x2sp.tile([P, SB], BF, tag="x2l")
                            nc.gpsimd.dma_start(x2t[:], x2_d[fsub * fb + sub, kt])
                            nc.vector.tensor_tensor(h2[:, kt, dsl], x2t[:], ab_pss[sub], ALU.mult)
                            nc.vector.tensor_tensor(h2[:, kt, dsl], h2[:, kt, dsl], cb_pss[sub], ALU.add)
                            if ln2_affine:
                                nc.vector.tensor_scalar(
                                    h2[:, kt, dsl], h2[:, kt, dsl],
                                    ln2_wb[:, kt:kt + 1], ln2_wb[:, nkt + kt:nkt + kt + 1],
                                    ALU.mult, ALU.add)
                    ut = utp.tile([P, nft, FB], BF, tag="ut")
                    for ft in range(nft):
                        w1f = w1p.tile([P, nkt * P], BF, tag="w1")
                        nc.sync.dma_start(w1f[:], w1_d[ft])
                        u_pss = [psmm.tile([P, SB], FP, tag="mm", name=f"ups{s}")
                                 for s in range(fsub)]
                        for kt in range(nkt):
                            for s2 in range(fsub):
                                nc.tensor.matmul(
                                    u_pss[s2][:], w1f[:, kt * P:(kt + 1) * P],
                                    h2[:, kt, s2 * SB:(s2 + 1) * SB],
                                    start=(kt == 0), stop=(kt == nkt - 1))
                        for s2 in range(fsub):
                            nc.scalar.activation(ut[:, ft, s2 * SB:(s2 + 1) * SB],
                                                 u_pss[s2][:], AF.Silu)
                    for hc in range(nkt):
                        w2h = w2p.tile([P, nft * P], BF, tag="w2")
                        nc.sync.dma_start(w2h[:], w2_d[hc])
                        y_pss = [psmm.tile([P, SB], FP, tag="mm", name=f"yps{s}")
                                 for s in range(fsub)]
                        for ft in range(nft):
                            for s2 in range(fsub):
                                nc.tensor.matmul(
                                    y_pss[s2][:], w2h[:, ft * P:(ft + 1) * P],
                                    ut[:, ft, s2 * SB:(s2 + 1) * SB],
                                    start=(ft == 0), stop=(ft == nft - 1))
                        for s2 in range(fsub):
                            gsl = slice(fb * FB + s2 * SB, fb * FB + (s2 + 1) * SB)
                            x2t = x2sp.tile([P, SB], BF, tag="x2r")
                            nc.gpsimd.dma_start(x2t[:], x2_d[fsub * fb + s2, hc])
                            yt = ysp.tile([P, SB], FP, tag="yt")
                            nc.vector.scalar_tensor_tensor(
                                yt[:], x2t[:], 1.0 / n_cores, y_pss[s2][:],
                                ALU.mult, ALU.add)
                            nc.gpsimd.dma_start(y_d[hc, :, gsl], yt[:])

                for fb in range(nfb):
                    ffn_block(fb)

    _lp.__exit__(None, None, None)
    nc.compile()
    return nc


# ---------------------------------------------------------------------------
# host side
# ---------------------------------------------------------------------------

def make_core_inputs(inputs, cfg, mask_mode, ln1_affine, ln2_affine):
    seq, hid, ffn = cfg["seq"], cfg["hid"], cfg["ffn"]
    n_cores, n_heads = cfg["n_cores"], cfg["n_heads"]
    d_nope, d_rope, d_v = cfg["d_nope"], cfg["d_rope"], cfg["d_v"]
    SB = cfg["sb"]
    half = d_rope // 2
    hpc = n_heads // n_cores
    nkt = hid // P
    nsb = seq // SB
    sbt = SB // P
    nskt = seq // P
    n_rope_ot = hpc * d_rope // P
    qo = hpc * d_nope // P + n_rope_ot
    dvc = hpc * d_v
    ndvt = dvc // P
    fpc = ffn // n_cores
    nft = fpc // P

    f32 = np.float32
    x = np.asarray(inputs["hidden_states"], dtype=f32)[0]        # [seq, hid]
    xt = np.ascontiguousarray(x.T.reshape(nkt, P, seq)).astype(BF_NP)

    inv = (1.0 / (10000.0 ** (np.arange(0, d_rope, 2, dtype=f32) / f32(d_rope)))).astype(f32)
    t = np.arange(seq, dtype=f32)
    freqs = t[:, None] * inv[None, :]
    cosT = np.cos(freqs).astype(f32).T                      # [half, seq]
    sinT = np.sin(freqs).astype(f32).T
    cos128 = np.ascontiguousarray(np.tile(cosT, (P // half, 1))).astype(BF_NP)
    sin128 = np.ascontiguousarray(
        np.tile(np.concatenate([-sinT, sinT], axis=0), (P // d_rope, 1)))
    rperm = np.zeros((P, P), dtype=f32)
    for blk in range(P // d_rope):
        b = blk * d_rope
        for i in range(half):
            rperm[b + half + i, b + i] = 1.0
            rperm[b + i, b + half + i] = 1.0

    common = {"xt": xt, "cos_t": cos128, "sin_t": sin128,
              "rperm": rperm.astype(BF_NP),
              "onc": np.ones((P, 1), dtype=BF_NP),
              "onr": np.ones((1, P), dtype=f32)}
    mask = np.asarray(inputs["attention_mask"], dtype=f32)[0, 0]  # [seq, seq]
    if mask_mode == "causal":
        m01 = np.zeros((nsb, P, sbt * SB), dtype=f32)
        qcol = np.arange(SB)
        for qb in range(nsb):
            for i in range(sbt):
                krow = (qb * sbt + i) * P + np.arange(P)[:, None]
                m01[qb, :, i * SB:(i + 1) * SB] = (qb * SB + qcol[None, :]) >= krow
        common["mask_t"] = m01.astype(BF_NP)
    elif mask_mode == "full":
        mT = np.ascontiguousarray(mask.T)                         # [sk, sq]
        m = np.empty((nskt, nsb, P, SB), dtype=f32)
        for tt in range(nskt):
            for qb in range(nsb):
                m[tt, qb] = mT[tt * P:(tt + 1) * P, qb * SB:(qb + 1) * SB]
        common["mask_t"] = m
    if ln1_affine:
        common["ln1_wb"] = np.ascontiguousarray(np.stack(
            [np.asarray(inputs["ln1_w"], f32), np.asarray(inputs["ln1_b"], f32)]
        ).reshape(2, nkt, P).transpose(2, 0, 1).reshape(P, 2 * nkt))
    if ln2_affine:
        common["ln2_wb"] = np.ascontiguousarray(np.stack(
            [np.asarray(inputs["ln2_w"], f32), np.asarray(inputs["ln2_b"], f32)]
        ).reshape(2, nkt, P).transpose(2, 0, 1).reshape(P, 2 * nkt))

    wq = np.asarray(inputs["w_q"], f32)
    wk = np.asarray(inputs["w_k"], f32)
    wv = np.asarray(inputs["w_v"], f32)
    wo = np.asarray(inputs["w_o"], f32)
    w1 = np.asarray(inputs["w1"], f32)
    w2 = np.asarray(inputs["w2"], f32)

    def batch_ot(w_rows):
        """[n*P out rows, hid] -> [n, P, nkt*P]: tile (ot)[p, kt*P+c] =
        w_rows[ot*P + c, kt*P + p] (transposed chunks, batched per out tile)."""
        n = w_rows.shape[0] // P
        return np.ascontiguousarray(
            w_rows.reshape(n, P, nkt, P).transpose(0, 3, 2, 1)
        ).reshape(n, P, nkt * P).astype(BF_NP)

    in_maps = []
    for c in range(n_cores):
        heads = range(c * hpc, (c + 1) * hpc)
        parts = []
        for w in (wq, wk):
            nope = np.concatenate([w[g * d_nope:(g + 1) * d_nope] for g in heads])
            rope = np.concatenate(
                [w[n_heads * d_nope + g * d_rope: n_heads * d_nope + (g + 1) * d_rope]
                 for g in heads])
            parts.append(np.concatenate([nope, rope]))
        wqk_t = batch_ot(np.concatenate(parts))                   # [2*qo, P, nkt*P]
        wv_c = np.concatenate([wv[g * d_v:(g + 1) * d_v] for g in heads])  # [dvc, hid]
        wv_t = np.ascontiguousarray(wv_c.T.reshape(nkt, P, dvc)).astype(BF_NP)
        wo_c = wo[:, c * dvc:(c + 1) * dvc]                       # [hid, dvc]
        wo_t = np.ascontiguousarray(
            wo_c.reshape(nkt, P, ndvt, P).transpose(0, 3, 2, 1)
        ).reshape(nkt, P, ndvt * P).astype(BF_NP)
        w1_c = w1[c * fpc:(c + 1) * fpc]                          # [fpc, hid]
        w1_t = np.ascontiguousarray(
            w1_c.reshape(nft, P, nkt, P).transpose(0, 3, 2, 1)
        ).reshape(nft, P, nkt * P).astype(BF_NP)
        w2_c = w2[:, c * fpc:(c + 1) * fpc]                       # [hid, fpc]
        w2_t = np.ascontiguousarray(
            w2_c.reshape(nkt, P, nft, P).transpose(0, 3, 2, 1)
        ).reshape(nkt, P, nft * P).astype(BF_NP)
        in_maps.append(dict(common, wqk_t=wqk_t, wv_t=wv_t, wo_t=wo_t,
                            w1_t=w1_t, w2_t=w2_t))
    return in_maps


def detect_mask_mode(mask, seq):
    if not mask.any():
        return "zero"
    iu = np.triu_indices(seq, 1)
    upper_blocked = bool((mask[iu] <= -1e8).all())
    il = np.tril_indices(seq)
    lower_zero = bool((mask[il] == 0).all())
    if upper_blocked and lower_zero:
        return "causal"
    return "full"


_BUILT = {}


def run_layer(inputs, cfg, trace=False):
    f32 = np.float32
    mask = np.asarray(inputs["attention_mask"], dtype=f32)[0, 0]
    mask_mode = detect_mask_mode(mask, cfg["seq"])
    ln1_affine = not ((np.asarray(inputs["ln1_w"]) == 1).all()
                     and (np.asarray(inputs["ln1_b"]) == 0).all())
    ln2_affine = not ((np.asarray(inputs["ln2_w"]) == 1).all()
                     and (np.asarray(inputs["ln2_b"]) == 0).all())
    key = (tuple(sorted(cfg.items())), mask_mode, ln1_affine, ln2_affine)
    if key not in _BUILT:
        _BUILT[key] = build_layer_kernel(cfg, mask_mode, ln1_affine, ln2_affine)
    nc = _BUILT[key]
    in_maps = make_core_inputs(inputs, cfg, mask_mode, ln1_affine, ln2_affine)
    res = run_bass_kernel_spmd(nc, in_maps, core_ids=list(range(cfg["n_cores"])),
                               trace=trace)
    acc = np.zeros((cfg["hid"], cfg["seq"]), dtype=np.float64)
    for c in range(cfg["n_cores"]):
        acc += res.results[c]["y_t"].reshape(cfg["hid"], cfg["seq"])
    out = acc.T.astype(f32)[None]
    return out, res


def kernel(**inputs):
    out, _ = run_layer(inputs, CFG_FULL)
    return out


# revision 21
# speedup vs baseline: 2.0409x; 1.0846x over previous
"""Fused tensor-parallel transformer layer for Trainium2 (8 NeuronCores).

Sharding: Megatron-style tensor parallel. Each core owns 4 heads of the
attention block (q/k/v projection rows, o_proj columns) and 1/8 of the FFN
hidden dim (w1 rows, w2 columns). LayerNorms are computed replicated on
every core. One on-device AllReduce (bf16, Shared output) joins the
attention block to the FFN block; the final residual sum is assembled on
the host from per-core partial outputs (each core adds x2/8 so the
partials sum to the answer).

v2 layout: all matmul operands are bf16 (fast weight load + half the HBM
traffic), weights are DMA'd in one batched transfer per output tile,
K/V stay resident in SBUF (no DRAM round trip), the causal mask is a 0/1
multiply on the vector engine, and DMA issue is spread across the sync /
scalar / gpsimd queues. Activations stay transposed ([hid, seq]) so every
matmul contracts over the partition dim with zero on-device transposes.
"""

import math
import ml_dtypes
import numpy as np

import concourse.bass as bass
import concourse.mybir as mybir
import concourse.tile as tile
from concourse import bacc
from concourse.bass_utils import run_bass_kernel_spmd

FP = mybir.dt.float32
BF = mybir.dt.bfloat16
P = 128
EPS = 1e-6
AF = mybir.ActivationFunctionType
ALU = mybir.AluOpType
BF_NP = ml_dtypes.bfloat16


def fr(ap):
    return ap.bitcast(mybir.dt.float32r)


CFG_FULL = dict(
    seq=2048, hid=4096, ffn=16384, n_cores=8, n_heads=32,
    d_nope=128, d_rope=64, d_v=128, sb=512, ss=1024, fb=1024,
)


def build_layer_kernel(cfg, mask_mode, ln1_affine, ln2_affine):
    """mask_mode: 'causal' (skip tiles above diag, 0/1-multiply diag tiles),
    'zero' (no mask at all), 'full' (additive mask everywhere)."""
    seq, hid, ffn = cfg["seq"], cfg["hid"], cfg["ffn"]
    n_cores, n_heads = cfg["n_cores"], cfg["n_heads"]
    d_nope, d_rope, d_v = cfg["d_nope"], cfg["d_rope"], cfg["d_v"]
    SB, SS, FB = cfg["sb"], cfg["ss"], cfg["fb"]
    half = d_rope // 2
    hpc = n_heads // n_cores              # heads per core
    nkt = hid // P                        # hid k-tiles
    nsb = seq // SB                       # 512-wide blocks (attn q / stage E)
    sbt = SB // P                         # sk tiles per 512 block
    nskt = seq // P                       # total sk tiles
    nss = seq // SS                       # projection super blocks
    psub = SS // SB                       # 512 sub blocks per super block
    n_rope_ot = hpc * d_rope // P         # rope o-tiles (2 heads each)
    qo = hpc * d_nope // P + n_rope_ot    # q/k o-tiles per core
    dvc = hpc * d_v                       # v cols per core
    ndvt = dvc // P                       # o_proj contraction tiles
    fpc = ffn // n_cores                  # ffn rows per core
    nft = fpc // P                        # f tiles per core
    nfb = seq // FB                       # ffn s-blocks
    fsub = FB // SB                       # 512 sub blocks per ffn block
    assert hpc % 2 == 0 and half == 32 and d_nope == P and d_v == P
    assert nfb == 2 and nss == 2 and psub == 2 and fsub == 2

    nc = bacc.Bacc(None, target_bir_lowering=False)

    xt_d = nc.dram_tensor("xt", [nkt, P, seq], BF, kind="ExternalInput")
    wqk_d = nc.dram_tensor("wqk_t", [2 * qo, P, nkt * P], BF, kind="ExternalInput")
    wv_d = nc.dram_tensor("wv_t", [nkt, P, dvc], BF, kind="ExternalInput")
    wo_d = nc.dram_tensor("wo_t", [nkt, P, ndvt * P], BF, kind="ExternalInput")
    w1_d = nc.dram_tensor("w1_t", [nft, P, nkt * P], BF, kind="ExternalInput")
    w2_d = nc.dram_tensor("w2_t", [nkt, P, nft * P], BF, kind="ExternalInput")
    cos_d = nc.dram_tensor("cos_t", [P, seq], BF, kind="ExternalInput")
    sin_d = nc.dram_tensor("sin_t", [P, seq], FP, kind="ExternalInput")
    rp_d = nc.dram_tensor("rperm", [P, P], BF, kind="ExternalInput")
    onc_d = nc.dram_tensor("onc", [P, 1], BF, kind="ExternalInput")
    onr_d = nc.dram_tensor("onr", [1, P], FP, kind="ExternalInput")
    if mask_mode == "causal":
        mask_d = nc.dram_tensor("mask_t", [nsb, P, sbt * SB], BF, kind="ExternalInput")
    elif mask_mode == "full":
        mask_d = nc.dram_tensor("mask_t", [nskt, nsb, P, SB], FP, kind="ExternalInput")
    else:
        mask_d = None
    ln1_d = nc.dram_tensor("ln1_wb", [P, 2 * nkt], FP, kind="ExternalInput") if ln1_affine else None
    ln2_d = nc.dram_tensor("ln2_wb", [P, 2 * nkt], FP, kind="ExternalInput") if ln2_affine else None

    ar_in = nc.dram_tensor("ar_in", [nsb, nkt, P, SB], BF)
    ar_out = nc.dram_tensor("ar_out", [nsb, nkt, P, SB], BF, addr_space="Shared")
    x2_d = nc.dram_tensor("x2t", [nsb, nkt, P, SB], BF)
    y_d = nc.dram_tensor("y_t", [nkt, P, seq], FP, kind="ExternalOutput")

    q_nope_scale = 1.0 / math.sqrt(d_nope)
    q_rope_scale = 1.0 / math.sqrt(d_rope)

    _lp = nc.allow_low_precision(
        reason="bf16 matmul operands; fp32 SBUF views bitcast to float32r")
    _lp.__enter__()
    with tile.TileContext(nc) as tc:
        with (
            tc.tile_pool(name="const", bufs=1) as constp,
            tc.tile_pool(name="stat2", bufs=1) as stat2p,
            tc.tile_pool(name="stats", bufs=1) as statp,
            tc.tile_pool(name="sqp", bufs=2) as sqp,
            tc.tile_pool(name="xep", bufs=2) as xep,
            tc.tile_pool(name="psmm", bufs=6, space="PSUM") as psmm,
            tc.tile_pool(name="psln", bufs=1, space="PSUM") as psln,
        ):
            ones_col = constp.tile([P, 1], BF)
            nc.sync.dma_start(ones_col[:], onc_d[:, :])
            ones_row = constp.tile([1, P], FP)
            nc.sync.dma_start(fr(ones_row[:]), fr(onr_d[:, :]))
            ones_row_bf = constp.tile([1, P], BF)
            nc.vector.tensor_copy(out=ones_row_bf[:], in_=ones_row[:])
            eps_t = constp.tile([1, 1], FP)
            nc.any.memset(eps_t[:], EPS)
            rperm_t = constp.tile([P, P], BF)
            nc.sync.dma_start(rperm_t[:], rp_d[:, :])
            if ln1_affine:
                ln1_wb = constp.tile([P, 2 * nkt], FP)
                nc.sync.dma_start(ln1_wb[:], ln1_d[:, :])
            if ln2_affine:
                ln2_wb = constp.tile([P, 2 * nkt], FP)
                nc.sync.dma_start(ln2_wb[:], ln2_d[:, :])
            a2_all = stat2p.tile([1, seq], BF, tag="a2")
            c2_all = stat2p.tile([1, seq], BF, tag="c2")

            def bcast(row_sbuf):
                """[1, n<=SB] sbuf row -> [P, n] psum via PE rank-1 matmul."""
                n = row_sbuf.shape[-1]
                ps = psmm.tile([P, SB], FP, tag="mm", name="bc")
                ps = ps[:, :n]
                if row_sbuf.dtype == BF:
                    nc.tensor.matmul(ps, ones_row_bf[:], row_sbuf, start=True, stop=True)
                else:
                    nc.tensor.matmul(ps, fr(ones_row[:]), fr(row_sbuf), start=True, stop=True)
                return ps

            def ln_stats(sum_ps, ssq_ps):
                mu = statp.tile([1, SB], FP, tag="mu")
                nc.scalar.activation(mu[:], sum_ps[:1, :], AF.Copy, scale=1.0 / hid)
                msq = statp.tile([1, SB], FP, tag="msq")
                nc.scalar.activation(msq[:], ssq_ps[:1, :], AF.Copy, scale=1.0 / hid)
                var = statp.tile([1, SB], FP, tag="var")
                nc.vector.tensor_tensor(var[:], mu[:], mu[:], ALU.mult)
                nc.vector.tensor_tensor(var[:], msq[:], var[:], ALU.subtract)
                std = statp.tile([1, SB], FP, tag="std")
                nc.scalar.activation(std[:], var[:], AF.Sqrt, bias=eps_t[:])
                rstd = statp.tile([1, SB], FP, tag="rstd")
                nc.vector.reciprocal(fr(rstd[:]), std[:])
                nmr = statp.tile([1, SB], FP, tag="nmr")
                nc.vector.tensor_tensor(fr(nmr[:]), mu[:], rstd[:], ALU.mult)
                nc.vector.tensor_scalar_mul(fr(nmr[:]), nmr[:], -1.0)
                return rstd, nmr

            def stage_e(sb):
                """x2 = x + attn_allreduce; write x2 (bf16); LN2 stats."""
                ssl = slice(sb * SB, (sb + 1) * SB)
                sum_ps = psln.tile([1, SB], FP, tag="lsum")
                ssq_ps = psln.tile([1, SB], FP, tag="lssq")
                for kt in range(nkt):
                    xe = xep.tile([P, SB], BF, tag="xe")
                    nc.sync.dma_start(xe[:], xt_d[kt, :, ssl])
                    are = xep.tile([P, SB], BF, tag="are")
                    nc.scalar.dma_start(are[:], ar_out[sb, kt])
                    x2t = xep.tile([P, SB], BF, tag="x2w")
                    nc.vector.tensor_tensor(x2t[:], xe[:], are[:], ALU.add)
                    nc.scalar.dma_start(x2_d[sb, kt], x2t[:])
                    sq = sqp.tile([P, SB], BF, tag="sq")
                    nc.vector.tensor_tensor(sq[:], x2t[:], x2t[:], ALU.mult)
                    nc.tensor.matmul(sum_ps[:], ones_col[:], x2t[:],
                                     start=(kt == 0), stop=(kt == nkt - 1))
                    nc.tensor.matmul(ssq_ps[:], ones_col[:], sq[:],
                                     start=(kt == 0), stop=(kt == nkt - 1))
                rstd, nmr = ln_stats(sum_ps, ssq_ps)
                nc.vector.tensor_copy(out=a2_all[:, ssl], in_=rstd[:])
                nc.vector.tensor_copy(out=c2_all[:, ssl], in_=nmr[:])

            # ---------------- attention block ----------------
            with (
                tc.tile_pool(name="ht", bufs=1) as htp,
                tc.tile_pool(name="qt", bufs=1) as qtp,
                tc.tile_pool(name="kall", bufs=1) as kallp,
                tc.tile_pool(name="vall", bufs=1) as vallp,
                tc.tile_pool(name="ctxp", bufs=1) as ctxp,
                tc.tile_pool(name="expp", bufs=3) as expp,
                tc.tile_pool(name="wqkp", bufs=2) as wqkp,
                tc.tile_pool(name="wvp", bufs=2) as wvp,
                tc.tile_pool(name="wop", bufs=2) as wop,
                tc.tile_pool(name="trig", bufs=1) as trigp,
                tc.tile_pool(name="maskp", bufs=1) as mp,
                tc.tile_pool(name="miscp", bufs=2) as miscp,
            ):
                k_all = kallp.tile([P, qo, seq], BF, tag="kall")
                v_all = vallp.tile([P, nskt, dvc], BF, tag="vall")

                def rope_apply(dest, raw, cs, sn):
                    """dest(bf16)/raw(bf16): [P, SB]; rows per 64-block: x1|x2.
                    out = raw*cos + swap(raw)*sin_signed, swap via PE perm."""
                    ps_sw = psmm.tile([P, SB], FP, tag="mm", name="swp")
                    nc.tensor.matmul(ps_sw[:], rperm_t[:], raw[:], start=True, stop=True)
                    m1 = miscp.tile([P, SB], FP, tag="mtmp", name="m1")
                    m2 = miscp.tile([P, SB], FP, tag="mtmp", name="m2")
                    nc.vector.tensor_tensor(m1[:], raw[:], cs, ALU.mult)
                    nc.vector.tensor_tensor(m2[:], ps_sw[:], sn, ALU.mult)
                    nc.vector.tensor_tensor(dest, m1[:], m2[:], ALU.add)

                def attn_superblock(ss):
                    ssl = slice(ss * SS, (ss + 1) * SS)
                    # --- stage A: load x tiles, LN1 stats, normalize in place
                    ht = htp.tile([P, nkt, SS], BF, tag="ht")
                    for kt in range(nkt):
                        nc.sync.dma_start(ht[:, kt, :], xt_d[kt, :, ssl])
                    for sub in range(psub):
                        dsl = slice(sub * SB, (sub + 1) * SB)
                        sum_ps = psln.tile([1, SB], FP, tag="lsum")
                        ssq_ps = psln.tile([1, SB], FP, tag="lssq")
                        for kt in range(nkt):
                            sq = sqp.tile([P, SB], BF, tag="sq")
                            nc.vector.tensor_tensor(sq[:], ht[:, kt, dsl], ht[:, kt, dsl], ALU.mult)
                            nc.tensor.matmul(sum_ps[:], ones_col[:], ht[:, kt, dsl],
                                             start=(kt == 0), stop=(kt == nkt - 1))
                            nc.tensor.matmul(ssq_ps[:], ones_col[:], sq[:],
                                             start=(kt == 0), stop=(kt == nkt - 1))
                        rstd, nmr = ln_stats(sum_ps, ssq_ps)
                        ab_ps = bcast(rstd[:])
                        cb_ps = bcast(nmr[:])
                        ab_bf = miscp.tile([P, SB], BF, tag="raw", name="ab_bf")
                        nc.scalar.activation(ab_bf[:], ab_ps[:], AF.Copy)
                        cb_bf = miscp.tile([P, SB], BF, tag="raw", name="cb_bf")
                        nc.scalar.activation(cb_bf[:], cb_ps[:], AF.Copy)
                        for kt in range(nkt):
                            nc.vector.tensor_tensor(ht[:, kt, dsl], ht[:, kt, dsl], ab_bf[:], ALU.mult)
                            nc.vector.tensor_tensor(ht[:, kt, dsl], ht[:, kt, dsl], cb_bf[:], ALU.add)
                            if ln1_affine:
                                nc.vector.tensor_scalar(
                                    ht[:, kt, dsl], ht[:, kt, dsl],
                                    ln1_wb[:, kt:kt + 1], ln1_wb[:, nkt + kt:nkt + kt + 1],
                                    ALU.mult, ALU.add)

                    # --- stage B: q/k/v projections for this super block ---
                    cs_ss = trigp.tile([P, SS], BF, tag="cos")
                    nc.sync.dma_start(cs_ss[:], cos_d[:, ssl])
                    sn_ss = trigp.tile([P, SS], FP, tag="sin")
                    nc.sync.dma_start(fr(sn_ss[:]), fr(sin_d[:, ssl]))
                    qt = qtp.tile([P, qo, SS], BF, tag="qt")
                    for ot in range(2 * qo):
                        wt = wqkp.tile([P, nkt * P], BF, tag="wqk")
                        nc.sync.dma_start(wt[:], wqk_d[ot])
                        pss = [psmm.tile([P, SB], FP, tag="mm", name=f"proj{s}")
                               for s in range(psub)]
                        for kt in range(nkt):
                            for s2 in range(psub):
                                nc.tensor.matmul(
                                    pss[s2][:], wt[:, kt * P:(kt + 1) * P],
                                    ht[:, kt, s2 * SB:(s2 + 1) * SB],
                                    start=(kt == 0), stop=(kt == nkt - 1))
                        is_q = ot < qo
                        o = ot % qo
                        is_rope = o >= qo - n_rope_ot
                        for s2 in range(psub):
                            gsl = slice(ss * SS + s2 * SB, ss * SS + (s2 + 1) * SB)
                            if is_q:
                                dest = qt[:, o, s2 * SB:(s2 + 1) * SB]
                                scale = q_rope_scale if is_rope else q_nope_scale
                            else:
                                dest = k_all[:, o, gsl]
                                scale = 1.0
                            if not is_rope:
                                nc.scalar.activation(dest, pss[s2][:], AF.Copy, scale=scale)
                            else:
                                raw = miscp.tile([P, SB], BF, tag="raw")
                                nc.scalar.activation(raw[:], pss[s2][:], AF.Copy, scale=scale)
                                dsl2 = slice(s2 * SB, (s2 + 1) * SB)
                                rope_apply(dest, raw[:], cs_ss[:, dsl2], sn_ss[:, dsl2])

                    for hf in range(2):
                        v_pss = [psmm.tile([P, dvc], FP, tag="mm", name=f"vps{i}")
                                 for i in range(4)]
                        for kt in range(nkt):
                            wvt = wvp.tile([P, dvc], BF, tag="wv")
                            nc.sync.dma_start(wvt[:], wv_d[kt])
                            for i in range(4):
                                sc = hf * 4 + i
                                nc.tensor.matmul(
                                    v_pss[i][:], ht[:, kt, sc * P:(sc + 1) * P], wvt[:],
                                    start=(kt == 0), stop=(kt == nkt - 1))
                        for i in range(4):
                            t_idx = ss * (SS // P) + hf * 4 + i
                            nc.scalar.activation(v_all[:, t_idx, :], v_pss[i][:], AF.Copy)

                    # --- stages C/D/E per 512-wide q-block ---
                    for qb in range(psub):
                        sb = ss * psub + qb
                        qsl = slice(qb * SB, (qb + 1) * SB)
                        t_max = (sb + 1) * sbt if mask_mode == "causal" else nskt
                        if mask_mode == "causal":
                            mt = mp.tile([P, sbt * SB], BF, tag="mask")
                            nc.sync.dma_start(mt[:], mask_d[sb])
                        ctxt = ctxp.tile([P, hpc, SB], BF, tag="ctx")
                        for h in range(hpc):
                            rot = qo - n_rope_ot + h // 2
                            rsl = slice(64 * (h % 2), 64 * (h % 2) + 64)
                            sum_ps = psmm.tile([1, SB], FP, tag="mm", name="smx")
                            ctx_ps = psmm.tile([P, SB], FP, tag="mm", name="ctxps")
                            for t in range(t_max):
                                tsl = slice(t * P, (t + 1) * P)
                                st_ps = psmm.tile([P, SB], FP, tag="mm", name="st")
                                nc.tensor.matmul(st_ps[:], k_all[:, h, tsl],
                                                 qt[:, h, qsl], start=True, stop=False)
                                nc.tensor.matmul(st_ps[:], k_all[rsl, rot, tsl],
                                                 qt[rsl, rot, qsl],
                                                 start=False, stop=True)
                                es = expp.tile([P, SB], BF, tag="es")
                                if mask_mode == "full":
                                    mtf = mp.tile([P, SB], FP, tag="maskf")
                                    nc.sync.dma_start(fr(mtf[:]), fr(mask_d[t, sb]))
                                    stf = miscp.tile([P, SB], FP, tag="stf")
                                    nc.vector.tensor_tensor(stf[:], st_ps[:], mtf[:], ALU.add)
                                    nc.scalar.activation(es[:], stf[:], AF.Exp)
                                else:
                                    nc.scalar.activation(es[:], st_ps[:], AF.Exp)
                                if mask_mode == "causal" and t >= sb * sbt:
                                    i = t - sb * sbt
                                    nc.vector.tensor_tensor(
                                        es[:], es[:], mt[:, i * SB:(i + 1) * SB], ALU.mult)
                                nc.tensor.matmul(sum_ps[:], ones_col[:], es[:],
                                                 start=(t == 0), stop=(t == t_max - 1))
                                nc.tensor.matmul(ctx_ps[:], v_all[:, t, h * P:(h + 1) * P],
                                                 es[:], start=(t == 0), stop=(t == t_max - 1))
                            sum_sb = statp.tile([1, SB], FP, tag="rec")
                            nc.scalar.activation(fr(sum_sb[:]), sum_ps[:1, :], AF.Copy)
                            rb_ps = bcast(sum_sb[:])
                            rbw = miscp.tile([P, SB], FP, tag="mtmp", name="rbw")
                            nc.vector.reciprocal(fr(rbw[:]), rb_ps[:])
                            nc.vector.tensor_tensor(ctxt[:, h, :], ctx_ps[:], rbw[:], ALU.mult)

                        # --- stage D: partial o_proj -> ar_in ---
                        for hc in range(nkt):
                            wot = wop.tile([P, ndvt * P], BF, tag="wo")
                            nc.sync.dma_start(wot[:], wo_d[hc])
                            o_ps = psmm.tile([P, SB], FP, tag="mm", name="ops")
                            for dvt in range(ndvt):
                                nc.tensor.matmul(o_ps[:], wot[:, dvt * P:(dvt + 1) * P],
                                                 ctxt[:, dvt, :],
                                                 start=(dvt == 0), stop=(dvt == ndvt - 1))
                            ao = miscp.tile([P, SB], BF, tag="ao")
                            nc.scalar.activation(ao[:], o_ps[:], AF.Copy)
                            nc.gpsimd.dma_start(ar_in[sb, hc], ao[:])
                        nc.gpsimd.collective_compute(
                            "AllReduce", ALU.add,
                            replica_groups=[list(range(n_cores))],
                            ins=[ar_in[sb].opt()], outs=[ar_out[sb].opt()])
                        if sb >= 1:
                            stage_e(sb - 1)

                for ss in range(nss):
                    attn_superblock(ss)

            # ---------------- FFN block ----------------
            with (
                tc.tile_pool(name="h2p", bufs=1) as h2p,
                tc.tile_pool(name="utp", bufs=1) as utp,
                tc.tile_pool(name="w1p", bufs=2) as w1p,
                tc.tile_pool(name="w2p", bufs=2) as w2p,
                tc.tile_pool(name="x2sp", bufs=3) as x2sp,
                tc.tile_pool(name="ysp", bufs=2) as ysp,
                tc.tile_pool(name="fbc", bufs=2) as fbc,
            ):
                def build_h2(fb):
                    ab_bfs, cb_bfs = [], []
                    for sub in range(fsub):
                        gsl = slice(fb * FB + sub * SB, fb * FB + (sub + 1) * SB)
                        ab_ps = bcast(a2_all[:, gsl])
                        cb_ps = bcast(c2_all[:, gsl])
                        ab_bf = fbc.tile([P, SB], BF, tag="fab", name="fab")
                        nc.scalar.activation(ab_bf[:], ab_ps[:], AF.Copy)
                        cb_bf = fbc.tile([P, SB], BF, tag="fcb", name="fcb")
                        nc.scalar.activation(cb_bf[:], cb_ps[:], AF.Copy)
                        ab_bfs.append(ab_bf)
                        cb_bfs.append(cb_bf)
                    h2 = h2p.tile([P, nkt, FB], BF, tag="h2")
                    for kt in range(nkt):
                        for sub in range(fsub):
                            dsl = slice(sub * SB, (sub + 1) * SB)
                            x2t = x2sp.tile([P, SB], BF, tag="x2l")
                            nc.gpsimd.dma_start(x2t[:], x2_d[fsub * fb + sub, kt])
                            nc.vector.tensor_tensor(h2[:, kt, dsl], x2t[:], ab_bfs[sub][:], ALU.mult)
                            nc.vector.tensor_tensor(h2[:, kt, dsl], h2[:, kt, dsl], cb_bfs[sub][:], ALU.add)
                            if ln2_affine:
                                nc.vector.tensor_scalar(
                                    h2[:, kt, dsl], h2[:, kt, dsl],
                                    ln2_wb[:, kt:kt + 1], ln2_wb[:, nkt + kt:nkt + kt + 1],
                                    ALU.mult, ALU.add)
                    return h2

                def u_phase(fb, h2):
                    ut = utp.tile([P, nft, FB], BF, tag="ut")
                    for ft in range(nft):
                        w1f = w1p.tile([P, nkt * P], BF, tag="w1")
                        nc.sync.dma_start(w1f[:], w1_d[ft])
                        u_pss = [psmm.tile([P, SB], FP, tag="mm", name=f"ups{s}")
                                 for s in range(fsub)]
                        for kt in range(nkt):
                            for s2 in range(fsub):
                                nc.tensor.matmul(
                                    u_pss[s2][:], w1f[:, kt * P:(kt + 1) * P],
                                    h2[:, kt, s2 * SB:(s2 + 1) * SB],
                                    start=(kt == 0), stop=(kt == nkt - 1))
                        for s2 in range(fsub):
                            nc.scalar.activation(ut[:, ft, s2 * SB:(s2 + 1) * SB],
                                                 u_pss[s2][:], AF.Silu)
                    return ut

                def y_phase(fb, ut):
                    for hc in range(nkt):
                        w2h = w2p.tile([P, nft * P], BF, tag="w2")
                        nc.sync.dma_start(w2h[:], w2_d[hc])
                        y_pss = [psmm.tile([P, SB], FP, tag="mm", name=f"yps{s}")
                                 for s in range(fsub)]
                        for ft in range(nft):
                            for s2 in range(fsub):
                                nc.tensor.matmul(
                                    y_pss[s2][:], w2h[:, ft * P:(ft + 1) * P],
                                    ut[:, ft, s2 * SB:(s2 + 1) * SB],
                                    start=(ft == 0), stop=(ft == nft - 1))
                        for s2 in range(fsub):
                            gsl = slice(fb * FB + s2 * SB, fb * FB + (s2 + 1) * SB)
                            x2t = x2sp.tile([P, SB], BF, tag="x2r")
                            nc.gpsimd.dma_start(x2t[:], x2_d[fsub * fb + s2, hc])
                            yt = ysp.tile([P, SB], FP, tag="yt")
                            nc.vector.scalar_tensor_tensor(
                                yt[:], x2t[:], 1.0 / n_cores, y_pss[s2][:],
                                ALU.mult, ALU.add)
                            nc.gpsimd.dma_start(y_d[hc, :, gsl], yt[:])

                # emission order interleaves block 1's LN-apply (vector) with
                # block 0's w2 matmuls (PE) so the PE never idles at the seam
                h2_0 = build_h2(0)
                ut_0 = u_phase(0, h2_0)
                stage_e(nsb - 1)
                h2_1 = build_h2(1)
                y_phase(0, ut_0)
                ut_1 = u_phase(1, h2_1)
                y_phase(1, ut_1)

    _lp.__exit__(None, None, None)
    nc.compile()
    return nc


# ---------------------------------------------------------------------------
# host side
# ---------------------------------------------------------------------------

def make_core_inputs(inputs, cfg, mask_mode, ln1_affine, ln2_affine):
    seq, hid, ffn = cfg["seq"], cfg["hid"], cfg["ffn"]
    n_cores, n_heads = cfg["n_cores"], cfg["n_heads"]
    d_nope, d_rope, d_v = cfg["d_nope"], cfg["d_rope"], cfg["d_v"]
    SB = cfg["sb"]
    half = d_rope // 2
    hpc = n_heads // n_cores
    nkt = hid // P
    nsb = seq // SB
    sbt = SB // P
    nskt = seq // P
    n_rope_ot = hpc * d_rope // P
    qo = hpc * d_nope // P + n_rope_ot
    dvc = hpc * d_v
    ndvt = dvc // P
    fpc = ffn // n_cores
    nft = fpc // P

    f32 = np.float32
    x = np.asarray(inputs["hidden_states"], dtype=f32)[0]        # [seq, hid]
    xt = np.ascontiguousarray(x.T.reshape(nkt, P, seq)).astype(BF_NP)

    inv = (1.0 / (10000.0 ** (np.arange(0, d_rope, 2, dtype=f32) / f32(d_rope)))).astype(f32)
    t = np.arange(seq, dtype=f32)
    freqs = t[:, None] * inv[None, :]
    cosT = np.cos(freqs).astype(f32).T                      # [half, seq]
    sinT = np.sin(freqs).astype(f32).T
    cos128 = np.ascontiguousarray(np.tile(cosT, (P // half, 1))).astype(BF_NP)
    sin128 = np.ascontiguousarray(
        np.tile(np.concatenate([-sinT, sinT], axis=0), (P // d_rope, 1)))
    rperm = np.zeros((P, P), dtype=f32)
    for blk in range(P // d_rope):
        b = blk * d_rope
        for i in range(half):
            rperm[b + half + i, b + i] = 1.0
            rperm[b + i, b + half + i] = 1.0

    common = {"xt": xt, "cos_t": cos128, "sin_t": sin128,
              "rperm": rperm.astype(BF_NP),
              "onc": np.ones((P, 1), dtype=BF_NP),
              "onr": np.ones((1, P), dtype=f32)}
    mask = np.asarray(inputs["attention_mask"], dtype=f32)[0, 0]  # [seq, seq]
    if mask_mode == "causal":
        m01 = np.zeros((nsb, P, sbt * SB), dtype=f32)
        qcol = np.arange(SB)
        for qb in range(nsb):
            for i in range(sbt):
                krow = (qb * sbt + i) * P + np.arange(P)[:, None]
                m01[qb, :, i * SB:(i + 1) * SB] = (qb * SB + qcol[None, :]) >= krow
        common["mask_t"] = m01.astype(BF_NP)
    elif mask_mode == "full":
        mT = np.ascontiguousarray(mask.T)                         # [sk, sq]
        m = np.empty((nskt, nsb, P, SB), dtype=f32)
        for tt in range(nskt):
            for qb in range(nsb):
                m[tt, qb] = mT[tt * P:(tt + 1) * P, qb * SB:(qb + 1) * SB]
        common["mask_t"] = m
    if ln1_affine:
        common["ln1_wb"] = np.ascontiguousarray(np.stack(
            [np.asarray(inputs["ln1_w"], f32), np.asarray(inputs["ln1_b"], f32)]
        ).reshape(2, nkt, P).transpose(2, 0, 1).reshape(P, 2 * nkt))
    if ln2_affine:
        common["ln2_wb"] = np.ascontiguousarray(np.stack(
            [np.asarray(inputs["ln2_w"], f32), np.asarray(inputs["ln2_b"], f32)]
        ).reshape(2, nkt, P).transpose(2, 0, 1).reshape(P, 2 * nkt))

    wq = np.asarray(inputs["w_q"], f32)
    wk = np.asarray(inputs["w_k"], f32)
    wv = np.asarray(inputs["w_v"], f32)
    wo = np.asarray(inputs["w_o"], f32)
    w1 = np.asarray(inputs["w1"], f32)
    w2 = np.asarray(inputs["w2"], f32)

    def batch_ot(w_rows):
        """[n*P out rows, hid] -> [n, P, nkt*P]: tile (ot)[p, kt*P+c] =
        w_rows[ot*P + c, kt*P + p] (transposed chunks, batched per out tile)."""
        n = w_rows.shape[0] // P
        return np.ascontiguousarray(
            w_rows.reshape(n, P, nkt, P).transpose(0, 3, 2, 1)
        ).reshape(n, P, nkt * P).astype(BF_NP)

    in_maps = []
    for c in range(n_cores):
        heads = range(c * hpc, (c + 1) * hpc)
        parts = []
        for w in (wq, wk):
            nope = np.concatenate([w[g * d_nope:(g + 1) * d_nope] for g in heads])
            rope = np.concatenate(
                [w[n_heads * d_nope + g * d_rope: n_heads * d_nope + (g + 1) * d_rope]
                 for g in heads])
            parts.append(np.concatenate([nope, rope]))
        wqk_t = batch_ot(np.concatenate(parts))                   # [2*qo, P, nkt*P]
        wv_c = np.concatenate([wv[g * d_v:(g + 1) * d_v] for g in heads])  # [dvc, hid]
        wv_t = np.ascontiguousarray(wv_c.T.reshape(nkt, P, dvc)).astype(BF_NP)
        wo_c = wo[:, c * dvc:(c + 1) * dvc]                       # [hid, dvc]
        wo_t = np.ascontiguousarray(
            wo_c.reshape(nkt, P, ndvt, P).transpose(0, 3, 2, 1)
        ).reshape(nkt, P, ndvt * P).astype(BF_NP)
        w1_c = w1[c * fpc:(c + 1) * fpc]                          # [fpc, hid]
        w1_t = np.ascontiguousarray(
            w1_c.reshape(nft, P, nkt, P).transpose(0, 3, 2, 1)
        ).reshape(nft, P, nkt * P).astype(BF_NP)
        w2_c = w2[:, c * fpc:(c + 1) * fpc]                       # [hid, fpc]
        w2_t = np.ascontiguousarray(
            w2_c.reshape(nkt, P, nft, P).transpose(0, 3, 2, 1)
        ).reshape(nkt, P, nft * P).astype(BF_NP)
        in_maps.append(dict(common, wqk_t=wqk_t, wv_t=wv_t, wo_t=wo_t,
                            w1_t=w1_t, w2_t=w2_t))
    return in_maps


def detect_mask_mode(mask, seq):
    if not mask.any():
        return "zero"
    iu = np.triu_indices(seq, 1)
    upper_blocked = bool((mask[iu] <= -1e8).all())
    il = np.tril_indices(seq)
    lower_zero = bool((mask[il] == 0).all())
    if upper_blocked and lower_zero:
        return "causal"
    return "full"


_BUILT = {}


def run_layer(inputs, cfg, trace=False):
    f32 = np.float32
    mask = np.asarray(inputs["attention_mask"], dtype=f32)[0, 0]
    mask_mode = detect_mask_mode(mask, cfg["seq"])
    ln1_affine = not ((np.asarray(inputs["ln1_w"]) == 1).all()
                     and (np.asarray(inputs["ln1_b"]) == 0).all())
    ln2_affine = not ((np.asarray(inputs["ln2_w"]) == 1).all()
                     and (np.asarray(inputs["ln2_b"]) == 0).all())
    key = (tuple(sorted(cfg.items())), mask_mode, ln1_affine, ln2_affine)
    if key not in _BUILT:
        _BUILT[key] = build_layer_kernel(cfg, mask_mode, ln1_affine, ln2_affine)
    nc = _BUILT[key]
    in_maps = make_core_inputs(inputs, cfg, mask_mode, ln1_affine, ln2_affine)
    res = run_bass_kernel_spmd(nc, in_maps, core_ids=list(range(cfg["n_cores"])),
                               trace=trace)
    acc = np.zeros((cfg["hid"], cfg["seq"]), dtype=np.float64)
    for c in range(cfg["n_cores"]):
        acc += res.results[c]["y_t"].reshape(cfg["hid"], cfg["seq"])
    out = acc.T.astype(f32)[None]
    return out, res


def kernel(**inputs):
    out, _ = run_layer(inputs, CFG_FULL)
    return out
